# revision 56
# baseline (speedup 1.0000x reference)
"""Trainium2 Bass kernel for the B-spline sampling problem.

Computes, for sample points s = 0..S-1 (arange(0, t_max, delta)):
    B = bspline_basis(s, knots, degree=3)   # Cox-de Boor with the torch-loop
                                            # stale-column quirk
    out = B @ c.T

Strategy (8 NeuronCores, sample axis sharded):
  * B is 4-banded per row and sample points are sorted, so each 128-sample
    block touches only a ~dozen consecutive basis columns.  Each block gets a
    static window of W=24 columns (host picks lo_b via searchsorted).
  * The Cox-de Boor recursion is run on-device in a gauged form
        Bh_d[j] = g_d[j] * B_d[j],  g_d[j] = g_{d-1}[j]*(k[j+d]-k[j])
    which makes the first term's coefficient exactly (t - k_j), so each
    degree is 2 scalar_tensor_tensor + 2 tensor_tensor DVE ops over all
    blocks at once.  The gauge is divided out of c on the host.
  * The "stale column" behavior of the reference (columns >= 2048-d are not
    updated at degree d) is reproduced with a narrow fix-up op pair over the
    last 3 blocks, driven by host tables (zero on cores it doesn't apply to).
  * Per block: PE-transpose Bh [128,W] -> [W,128], copy to SBUF (ScalarE),
    then a PE matmul [W,128]^T @ [W,512] accumulating in PSUM, DMA'd
    straight from PSUM to DRAM.
  * Tables that are constant across partitions are stored as single rows in
    DRAM and broadcast to 128 partitions with a partition-step-0 DMA.

Host work is O(K) table prep plus one searchsorted over the sample grid
(window/span metadata); all O(S*DIM) math runs on the device.
"""

import numpy as np

DEGREE = 3
W = 24            # basis-column window per 128-sample block (recursion)
WG = 32           # padded window stride for the GEMM (32-partition packing)
VW = W + 4        # knot table width (reads up to w + DEGREE + 1)
BLK = 128         # samples per block (partition dim)
NCORE = 8
DIM = 512
NKNOT = 2048      # basis columns
S_TOTAL = 32768
NBLK = S_TOTAL // BLK // NCORE      # 32 blocks per core
SC = S_TOTAL // NCORE               # 4096 samples per core
NCHUNK = 4
CB = NBLK // NCHUNK                 # 8 blocks per chunk
CF = CB * W                         # chunk free width (192)
CFP = CF + 1                        # +1 zero pad col per chunk
CG = CB * WG                        # chunk width in the padded GEMM layout
FIXNB = 3                           # stale-col fix region: last 3 blocks
FIXW = FIXNB * W
GJ = 4                              # blocks per transpose/matmul group
NGRP = CB // GJ                     # groups per chunk (2)

f32 = np.float32
USE_F32R = True                     # PE matmul in float32r fast mode


# ----------------------------------------------------------------------------
# pure-numpy reference fallback (general shapes; exact port of the jax code)
# ----------------------------------------------------------------------------
def _numpy_reference(t, c, delta):
    t = np.asarray(t, np.float32)
    c = np.asarray(c, np.float32)
    knots = np.concatenate([np.zeros(DEGREE + 1, np.float32), t])
    t_max = float(t[-1])
    sp = np.arange(0.0, t_max, float(delta), dtype=np.float32)
    n_knots = knots.shape[0] - DEGREE - 1
    B = ((knots[None, :n_knots] <= sp[:, None]) &
         (sp[:, None] < knots[None, 1:n_knots + 1])).astype(np.float32)
    for d in range(1, DEGREE + 1):
        n = n_knots - d
        den1 = knots[d:d + n] - knots[:n]
        den2 = knots[d + 1:d + 1 + n] - knots[1:1 + n]
        s1 = np.where(den1 == 0, 1.0, den1).astype(np.float32)
        s2 = np.where(den2 == 0, 1.0, den2).astype(np.float32)
        w1 = np.where(den1 == 0, 0.0, (sp[:, None] - knots[None, :n]) / s1)
        w2 = np.where(den2 == 0, 0.0,
                      (knots[None, d + 1:d + 1 + n] - sp[:, None]) / s2)
        B = B.copy()
        B[:, :n] = (w1 * B[:, :n] + w2 * B[:, 1:n + 1]).astype(np.float32)
    return sp, (B @ c.T).astype(np.float32)


# ----------------------------------------------------------------------------
# host-side table prep
# ----------------------------------------------------------------------------
def _host_prep(t, c):
    knots = np.concatenate([np.zeros(DEGREE + 1, np.float32),
                            np.asarray(t, np.float32)])
    k64 = knots.astype(np.float64)
    NB = NCORE * NBLK

    s = np.arange(S_TOTAL, dtype=np.float64)
    span = np.searchsorted(k64, s, side="right") - 1
    spans = span.reshape(NB, BLK)
    smin = spans.min(1)
    smax = spans.max(1)

    lo = np.clip(smin - 4, 0, NKNOT - W).astype(np.int64)
    if not (lo + W - 1 >= np.minimum(smax, NKNOT - 1)).all():
        raise ValueError("window too narrow for this knot distribution")
    if not (lo[1:] <= smin[1:] - 4).all():
        raise ValueError("left margin violated")

    # gauges (fp64): g[d, j] = g[d-1, j] * (k[j+d]-k[j]) with 0-gap -> 1
    NJ = NKNOT + 8
    k64x = np.concatenate([k64, np.full(8, k64[-1])])
    g = np.ones((DEGREE + 1, NJ), np.float64)
    for d in range(1, DEGREE + 1):
        kj = np.arange(NJ)
        den = k64x[np.minimum(kj + d, NJ + 3)] - k64x[kj]
        g[d] = g[d - 1] * np.where(den == 0.0, 1.0, den)

    C = 128.0 * np.arange(NB, dtype=np.float64) + 64.0
    c64 = np.asarray(c, np.float64)

    maps = []
    for m in range(NCORE):
        gb = np.arange(m * NBLK, (m + 1) * NBLK)
        lob = lo[gb]
        Cb = C[gb]

        jj = lob[:, None] + np.arange(VW)[None, :]
        nkx = (Cb[:, None] - k64[np.minimum(jj, NKNOT + 3)]).astype(f32)

        a2 = np.zeros((DEGREE, NBLK, W), f32)
        kff = np.zeros((DEGREE, 2, FIXW), f32)
        for d in range(1, DEGREE + 1):
            j = lob[:, None] + np.arange(W)[None, :]
            den2 = (k64[np.minimum(j + d + 1, NKNOT + 3)]
                    - k64[np.minimum(j + 1, NKNOT + 3)])
            frozen = j >= NKNOT - d
            with np.errstate(divide="ignore"):
                val = -g[d, j] / (np.where(den2 == 0, 1.0, den2) * g[d - 1, j + 1])
            a2[d - 1] = np.where((den2 == 0) | frozen, 0.0, val).astype(f32)

            fb = np.arange(NBLK - FIXNB, NBLK)
            jf = lob[fb][:, None] + np.arange(W)[None, :]
            fz = jf >= NKNOT - d
            kff[d - 1, 1] = fz.astype(f32).reshape(FIXW)
            kff[d - 1, 0] = np.where(
                fz, k64[np.minimum(jf + d, NKNOT + 3)] - C[gb[fb]][:, None], 0.0
            ).astype(f32).reshape(FIXW)
            jall = lob[:NBLK - FIXNB][:, None] + np.arange(W)[None, :]
            if (jall >= NKNOT - d).any():
                raise ValueError("frozen cols outside fix region")
            den1 = k64[np.minimum(j + d, NKNOT + 3)] - k64[j]
            if ((den1 == 0) & (j > 2)).any():
                raise ValueError("tied interior knots unsupported")

        sp_ = spans[gb]
        Q = (sp_ - lob[:, None]).astype(np.int64)
        Q = np.where((Q < 0) | (Q >= W), -1000, Q)
        qtab = np.ascontiguousarray(Q.T.astype(f32))           # [BLK, NBLK]

        # gauged coefficients, padded to WG rows per block; rows W..WG-1 are
        # zero so garbage in the matching (unwritten) lhsT rows cancels
        j = (lob[:, None] + np.arange(W)[None, :]).reshape(-1)
        cw = (c64[:, j] / g[DEGREE, j]).T.astype(f32)          # [NBLK*W, DIM]
        ctile = np.zeros((NBLK * WG, DIM), f32)
        ctile.reshape(NBLK, WG, DIM)[:, :W, :] = cw.reshape(NBLK, W, DIM)

        widx = np.tile(np.arange(W, dtype=f32), NBLK)
        # flat per-degree tables so every DVE operand is a 2-D contiguous AP
        # (3-D/strided APs eat the walrus per-instruction sync budget).
        # Order groups tables by first use so the DMA pieces pipeline:
        # [widx, negk1 | negk2_1, a2_1 | negk2_2, a2_2 | negk2_3, a2_3 | kff]
        negk1 = nkx[:, 0:W].reshape(-1)
        negk2 = [nkx[:, d + 1:d + 1 + W].reshape(-1) for d in (1, 2, 3)]
        a2f = [a2[d - 1].reshape(-1) for d in (1, 2, 3)]
        tabs = np.concatenate(
            [widx, negk1,
             negk2[0], a2f[0], negk2[1], a2f[1], negk2[2], a2f[2],
             kff.reshape(-1)]
        ).astype(f32).reshape(1, -1)
        ucol = (np.arange(BLK, dtype=f32) - 64.0).reshape(BLK, 1)
        pcol = np.ascontiguousarray(np.concatenate([ucol, qtab], axis=1))

        maps.append({
            "ctile": ctile,
            "tabs": tabs,
            "pcol": pcol,
        })
    return maps


# ----------------------------------------------------------------------------
# device program
# ----------------------------------------------------------------------------
def _build_nc():
    from contextlib import ExitStack

    import concourse.bass as bass
    import concourse.mybir as mybir
    import concourse.tile as tile
    from concourse import bacc
    from concourse.masks import make_identity

    FP = mybir.dt.float32
    AL = mybir.AluOpType

    WF = NBLK * W
    TABW = 8 * WF + DEGREE * 2 * FIXW

    FPR = mybir.dt.float32r if USE_F32R else FP

    nc = bacc.Bacc()
    y_d = nc.dram_tensor("y", [SC, DIM], FP, kind="ExternalOutput")
    ctile_d = nc.dram_tensor("ctile", [NBLK * WG, DIM], FPR, kind="ExternalInput")
    tabs_d = nc.dram_tensor("tabs", [1, TABW], FP, kind="ExternalInput")
    pcol_d = nc.dram_tensor("pcol", [BLK, 1 + NBLK], FP, kind="ExternalInput")

    with ExitStack() as ctx:
        tc = ctx.enter_context(tile.TileContext(nc))
        const = ctx.enter_context(tc.tile_pool(name="const", bufs=1))
        bwide = ctx.enter_context(tc.tile_pool(name="bwide", bufs=1))
        b3p = ctx.enter_context(tc.tile_pool(name="b3", bufs=NCHUNK + 1))
        w2tp = ctx.enter_context(tc.tile_pool(name="w2tp", bufs=1))
        tmpp = ctx.enter_context(tc.tile_pool(name="tmpp", bufs=2))
        lhsp = ctx.enter_context(tc.tile_pool(name="lhsp", bufs=2 * NCHUNK))
        outp = ctx.enter_context(tc.tile_pool(name="outp", bufs=NBLK))
        pst = ctx.enter_context(tc.tile_pool(name="pst", bufs=2, space="PSUM"))
        psmm = ctx.enter_context(tc.tile_pool(name="psmm", bufs=5, space="PSUM"))

        ident = const.tile([BLK, BLK], FP)
        make_identity(nc, ident)

        # per-partition columns: u' and the span index for the one-hot
        pcol = const.tile([BLK, 1 + NBLK], FP, tag="pcol")
        nc.sync.dma_start(pcol, pcol_d[:, :])
        ucol = pcol[:, 0:1]
        qtab = pcol[:, 1:1 + NBLK]

        # per-column tables, broadcast-DMA'd in pieces ordered by first use;
        # later pieces are emitted inside the chunk loop so the first ones
        # get the DMA queues to themselves
        tabsb = const.tile([BLK, TABW], FP, tag="tabsb")

        def load_piece(off, ln):
            nc.sync.dma_start(
                tabsb[:, off:off + ln],
                tabs_d[:, off:off + ln].to_broadcast([BLK, ln]))

        load_piece(0, WF)                  # widx (gates the init one-hot)
        load_piece(WF, WF)                 # negk1
        load_piece(2 * WF, 2 * WF)         # d1 tables

        def tabrow(i):  # flat [BLK, WF] table row i
            return tabsb[:, i * WF:(i + 1) * WF]

        widx = tabrow(0)
        negk1 = tabrow(1)
        negk2 = [tabrow(2 * d) for d in (1, 2, 3)]      # negk2[d-1]
        a2 = [tabrow(2 * d + 1) for d in (1, 2, 3)]     # a2[d-1]
        KO = 8 * WF
        kf = [tabsb[:, KO + (d - 1) * 2 * FIXW:][:, 0:FIXW] for d in (1, 2, 3)]
        ff = [tabsb[:, KO + (d - 1) * 2 * FIXW + FIXW:][:, 0:FIXW]
              for d in (1, 2, 3)]

        # coefficient windows, per chunk: 4 blocks stacked on partitions
        # (partition 32*j + w holds block 4*g+j, col w) so the 4 matmuls of
        # a group can run row-packed at tile_position=(32j, 0).  The DMAs
        # are emitted inside the chunk loop so the startup table DMAs get
        # the DMA queues first.
        csb = [None] * NCHUNK
        ctile3 = ctile_d[:, :].rearrange("(g j w) d -> (j w) g d", j=GJ, w=WG)

        # gauged-basis ping/pong, [128, NCHUNK*(CF+1)] with a zero pad col
        # per chunk (never written after memset; serves the last shift read)
        wideA = bwide.tile([BLK, NCHUNK * CFP], FP, tag="wideA")
        wideB = bwide.tile([BLK, NCHUNK * CFP], FP, tag="wideB")
        # only the per-chunk pad cols need zeroing; the rest is written
        for wt in (wideA, wideB):
            nc.gpsimd.memset(
                wt[:, :].rearrange("p (c f) -> p c f", f=CFP)[:, :, CF:CFP],
                0.0)

        # PE-side absorber: soak up the gpsimd(identity) wait once
        psd = ctx.enter_context(tc.tile_pool(name="psd", bufs=1, space="PSUM"))
        scrp = psd.tile([BLK, BLK], FP, tag="scrp")
        nc.tensor.transpose(scrp[:, :], ident[:, :], ident[:, :])

        def wslice(tile_, c, shift=0):   # flat [BLK, CF] chunk view
            base = c * CFP + shift
            return tile_[:, base:base + CF]

        def pslice(tile_, pr, shift=0):  # paired [BLK, 2, CF] chunk-pair view
            base = pr * 2 * CFP
            return tile_[:, base:base + 2 * CFP].rearrange(
                "p (c f) -> p c f", c=2)[:, :, shift:shift + CF]

        def ptab(tab, pr):               # paired table view [BLK, 2, CF]
            return tab[:, pr * 2 * CF:(pr + 1) * 2 * CF].rearrange(
                "p (c f) -> p c f", c=2)

        # ---- pair-major: recursion for a chunk pair, then its GEMM ----
        for pr in range(2):
            for ci in range(2):
                cidx = 2 * pr + ci
                csl = slice(cidx * CF, (cidx + 1) * CF)
                bsl = slice(cidx * CB, (cidx + 1) * CB)
                nc.vector.tensor_tensor(
                    wslice(wideA, cidx).rearrange("p (b w) -> p b w", w=W),
                    widx[:, csl].rearrange("p (b w) -> p b w", w=W),
                    qtab[:, bsl, None].to_broadcast([BLK, CB, W]),
                    AL.is_equal)
            src_, dst = wideA, wideB
            for d in (1, 2):
                out = pslice(dst, pr)
                nc.vector.scalar_tensor_tensor(
                    out, ptab(negk1, pr), ucol, pslice(src_, pr),
                    AL.add, AL.mult)
                w2t = w2tp.tile([BLK, 2, CF], FP, tag="w2t")
                nc.vector.scalar_tensor_tensor(
                    w2t[:, :, :], ptab(negk2[d - 1], pr), ucol,
                    ptab(a2[d - 1], pr), AL.add, AL.mult)
                nc.vector.tensor_tensor(
                    w2t[:, :, :], w2t[:, :, :], pslice(src_, pr, shift=1),
                    AL.mult)
                nc.vector.tensor_tensor(out, out, w2t[:, :, :], AL.add)
                if pr == 0:
                    if d == 1:
                        load_piece(4 * WF, 2 * WF)   # d2 tables
                    else:
                        load_piece(6 * WF, 2 * WF)   # d3 tables
                        load_piece(8 * WF, DEGREE * 2 * FIXW)  # kff
                # stale-column fix for degree d (last chunk's tail only)
                if pr == 1:
                    ntmp = tmpp.tile([BLK, FIXW], FP, tag="ntmp")
                    nc.vector.scalar_tensor_tensor(
                        ntmp[:, :], kf[d - 1], ucol,
                        wslice(src_, 3)[:, CF - FIXW:CF],
                        AL.subtract, AL.mult)
                    nc.vector.tensor_tensor(
                        ntmp[:, :], ntmp[:, :], ff[d - 1], AL.mult)
                    fout = out[:, 1, CF - FIXW:CF]
                    nc.vector.tensor_tensor(fout, fout, ntmp[:, :], AL.add)
                src_, dst = dst, src_

            for ci in range(2):
                cidx = 2 * pr + ci
                ct = const.tile([BLK, NGRP, DIM], FPR, tag=f"csb{cidx}")
                nc.sync.dma_start(
                    ct, ctile3[:, cidx * NGRP:(cidx + 1) * NGRP, :])
                csb[cidx] = ct
                nc.tensor.matmul(
                    scrp[:, 0:1], ct[:, 0, 0:BLK].bitcast(FP),
                    ct[:, 0, 0:1].bitcast(FP), start=True, stop=True)

            # d3 (into the padded GEMM layout) + GEMM for this pair's chunks
            for ci in range(2):
                cidx = 2 * pr + ci
                csl = slice(cidx * CF, (cidx + 1) * CF)
                b3t = b3p.tile([BLK, CG], FP, tag="b3")
                b3v = b3t[:, :].rearrange("p (b wg) -> p b wg", wg=WG)
                nc.gpsimd.memset(b3v[:, :, W:WG], 0.0)
                out = b3v[:, :, 0:W]

                def f3(ap):
                    return ap.rearrange("p (b w) -> p b w", w=W)

                nc.vector.scalar_tensor_tensor(
                    out, f3(negk1[:, csl]), ucol, f3(wslice(wideA, cidx)),
                    AL.add, AL.mult)
                w2t = w2tp.tile([BLK, CF], FP, tag="w2t3")
                nc.vector.scalar_tensor_tensor(
                    w2t[:, :], negk2[2][:, csl], ucol, a2[2][:, csl],
                    AL.add, AL.mult)
                nc.vector.tensor_tensor(
                    w2t[:, :], w2t[:, :], wslice(wideA, cidx, shift=1),
                    AL.mult)
                nc.vector.tensor_tensor(out, out, f3(w2t[:, :]), AL.add)
                if cidx == NCHUNK - 1:
                    ntmp = tmpp.tile([BLK, FIXW], FP, tag="ntmp")
                    nc.vector.scalar_tensor_tensor(
                        ntmp[:, :], kf[2], ucol,
                        wslice(wideA, cidx)[:, CF - FIXW:CF],
                        AL.subtract, AL.mult)
                    nc.vector.tensor_tensor(
                        ntmp[:, :], ntmp[:, :], ff[2], AL.mult)
                    fout = out[:, CB - FIXNB:CB, :]
                    nc.vector.tensor_tensor(
                        fout, fout, f3(ntmp[:, :]), AL.add)

                for gg in range(NGRP):
                    tps = pst.tile([BLK, BLK], FP, tag="tps")
                    nc.tensor.transpose(
                        tps[:, :], b3t[:, gg * GJ * WG:(gg + 1) * GJ * WG],
                        ident[:, :])
                    lh4 = lhsp.tile([BLK, BLK], FPR, tag="lh4")
                    nc.scalar.copy(lh4[:, :], tps[:, :])
                    for j in range(GJ):
                        gb = cidx * CB + gg * GJ + j
                        ps = psmm.tile([BLK, DIM], FP, tag="ps")
                        nc.tensor.matmul(
                            ps[:, :], lh4[32 * j:32 * (j + 1), :],
                            csb[cidx][32 * j:32 * (j + 1), gg, :],
                            start=True, stop=True, tile_position=(32 * j, 0))
                        ob = outp.tile([BLK, DIM], FP, tag="ob")
                        if pr == 1 and j % 2 == 0:
                            # DVE is past the recursion by now; share load
                            nc.vector.tensor_copy(ob[:, :], ps[:, :])
                        else:
                            nc.scalar.copy(ob[:, :], ps[:, :])
                        nc.sync.dma_start(y_d[gb * BLK:(gb + 1) * BLK, :],
                                          ob[:, :])

    nc.compile()
    return nc


_NC_CACHE = {}


def _get_nc():
    if "nc" not in _NC_CACHE:
        _NC_CACHE["nc"] = _build_nc()
    return _NC_CACHE["nc"]


# ----------------------------------------------------------------------------
# entry point
# ----------------------------------------------------------------------------
def kernel(t, c, delta):
    t = np.asarray(t, np.float32)
    c = np.asarray(c, np.float32)
    d = np.asarray(delta).reshape(()).item()

    t_max = float(t[-1])
    sample_points = np.arange(0.0, t_max, float(d), dtype=np.float32)
    S = sample_points.shape[0]

    if (S != S_TOTAL or t.shape != (NKNOT,) or c.shape != (DIM, NKNOT)):
        return _numpy_reference(t, c, d)

    try:
        maps = _host_prep(t, c)
    except ValueError:
        # pathological knot layout for the windowed scheme
        return _numpy_reference(t, c, d)

    from concourse.bass_utils import run_bass_kernel_spmd

    nc = _get_nc()
    res = run_bass_kernel_spmd(nc, maps, core_ids=list(range(NCORE)))
    out = np.concatenate([res.results[m]["y"] for m in range(NCORE)], axis=0)
    return sample_points, out


if __name__ == "__main__":
    import jax
    jax.config.update("jax_platforms", "cpu")
    rng = np.random.default_rng(0)
    # smoke-test the numpy fallback against itself via host tables + model
    print("kernel.py loaded OK")


# revision 57
# speedup vs baseline: 1.0981x; 1.0981x over previous
"""Trainium2 Bass kernel for the B-spline sampling problem.

Computes, for sample points s = 0..S-1 (arange(0, t_max, delta)):
    B = bspline_basis(s, knots, degree=3)   # Cox-de Boor with the torch-loop
                                            # stale-column quirk
    out = B @ c.T

Strategy (8 NeuronCores, sample axis sharded):
  * B is 4-banded per row and sample points are sorted, so each 128-sample
    block touches only a ~dozen consecutive basis columns.  Each block gets a
    static window of W=24 columns (host picks lo_b via searchsorted).
  * The Cox-de Boor recursion is run on-device in a gauged form
        Bh_d[j] = g_d[j] * B_d[j],  g_d[j] = g_{d-1}[j]*(k[j+d]-k[j])
    which makes the first term's coefficient exactly (t - k_j), so each
    degree is 2 scalar_tensor_tensor + 2 tensor_tensor DVE ops over all
    blocks at once.  The gauge is divided out of c on the host.
  * The "stale column" behavior of the reference (columns >= 2048-d are not
    updated at degree d) is reproduced with a narrow fix-up op pair over the
    last 3 blocks, driven by host tables (zero on cores it doesn't apply to).
  * GEMM: degree-3 results are written into a WG=32-padded layout; per group
    of 4 blocks one PE transpose [128,128] -> PSUM, one ScalarE copy to SBUF
    (rounding to float32r), then 4 row-packed float32r matmuls at
    tile_position=(32j,0) against 32-partition-stacked coefficient tiles.
    PSUM outputs are evacuated by ScalarE/VectorE and DMA'd to DRAM.
  * Tables that are constant across partitions are stored as single rows in
    DRAM and broadcast to 128 partitions with a partition-step-0 DMA, in
    pieces ordered/staggered by first use.  Oversized semaphore-wait sets
    are legalized by bacc's generate_event_semaphores.

Host work is O(K) table prep plus one searchsorted over the sample grid
(window/span metadata); all O(S*DIM) math runs on the device.
"""

import numpy as np

DEGREE = 3
W = 24            # basis-column window per 128-sample block (recursion)
WG = 32           # padded window stride for the GEMM (32-partition packing)
VW = W + 4        # knot table width (reads up to w + DEGREE + 1)
BLK = 128         # samples per block (partition dim)
NCORE = 8
DIM = 512
NKNOT = 2048      # basis columns
S_TOTAL = 32768
NBLK = S_TOTAL // BLK // NCORE      # 32 blocks per core
SC = S_TOTAL // NCORE               # 4096 samples per core
NCHUNK = 4
CB = NBLK // NCHUNK                 # 8 blocks per chunk
CF = CB * W                         # chunk free width (192)
CFP = CF + 1                        # +1 zero pad col per chunk
CG = CB * WG                        # chunk width in the padded GEMM layout
FIXNB = 3                           # stale-col fix region: last 3 blocks
FIXW = FIXNB * W
GJ = 4                              # blocks per transpose/matmul group
NGRP = CB // GJ                     # groups per chunk (2)

f32 = np.float32
USE_F32R = True                     # PE matmul in float32r fast mode


# ----------------------------------------------------------------------------
# pure-numpy reference fallback (general shapes; exact port of the jax code)
# ----------------------------------------------------------------------------
def _numpy_reference(t, c, delta):
    t = np.asarray(t, np.float32)
    c = np.asarray(c, np.float32)
    knots = np.concatenate([np.zeros(DEGREE + 1, np.float32), t])
    t_max = float(t[-1])
    sp = np.arange(0.0, t_max, float(delta), dtype=np.float32)
    n_knots = knots.shape[0] - DEGREE - 1
    B = ((knots[None, :n_knots] <= sp[:, None]) &
         (sp[:, None] < knots[None, 1:n_knots + 1])).astype(np.float32)
    for d in range(1, DEGREE + 1):
        n = n_knots - d
        den1 = knots[d:d + n] - knots[:n]
        den2 = knots[d + 1:d + 1 + n] - knots[1:1 + n]
        s1 = np.where(den1 == 0, 1.0, den1).astype(np.float32)
        s2 = np.where(den2 == 0, 1.0, den2).astype(np.float32)
        w1 = np.where(den1 == 0, 0.0, (sp[:, None] - knots[None, :n]) / s1)
        w2 = np.where(den2 == 0, 0.0,
                      (knots[None, d + 1:d + 1 + n] - sp[:, None]) / s2)
        B = B.copy()
        B[:, :n] = (w1 * B[:, :n] + w2 * B[:, 1:n + 1]).astype(np.float32)
    return sp, (B @ c.T).astype(np.float32)


# ----------------------------------------------------------------------------
# host-side table prep
# ----------------------------------------------------------------------------
def _host_prep(t, c):
    knots = np.concatenate([np.zeros(DEGREE + 1, np.float32),
                            np.asarray(t, np.float32)])
    k64 = knots.astype(np.float64)
    NB = NCORE * NBLK

    s = np.arange(S_TOTAL, dtype=np.float64)
    span = np.searchsorted(k64, s, side="right") - 1
    spans = span.reshape(NB, BLK)
    smin = spans.min(1)
    smax = spans.max(1)

    lo = np.clip(smin - 4, 0, NKNOT - W).astype(np.int64)
    if not (lo + W - 1 >= np.minimum(smax, NKNOT - 1)).all():
        raise ValueError("window too narrow for this knot distribution")
    if not (lo[1:] <= smin[1:] - 4).all():
        raise ValueError("left margin violated")

    # gauges (fp64): g[d, j] = g[d-1, j] * (k[j+d]-k[j]) with 0-gap -> 1
    NJ = NKNOT + 8
    k64x = np.concatenate([k64, np.full(8, k64[-1])])
    g = np.ones((DEGREE + 1, NJ), np.float64)
    for d in range(1, DEGREE + 1):
        kj = np.arange(NJ)
        den = k64x[np.minimum(kj + d, NJ + 3)] - k64x[kj]
        g[d] = g[d - 1] * np.where(den == 0.0, 1.0, den)

    C = 128.0 * np.arange(NB, dtype=np.float64) + 64.0
    c64 = np.asarray(c, np.float64)

    maps = []
    for m in range(NCORE):
        gb = np.arange(m * NBLK, (m + 1) * NBLK)
        lob = lo[gb]
        Cb = C[gb]

        jj = lob[:, None] + np.arange(VW)[None, :]
        nkx = (Cb[:, None] - k64[np.minimum(jj, NKNOT + 3)]).astype(f32)

        a2 = np.zeros((DEGREE, NBLK, W), f32)
        kff = np.zeros((DEGREE, 2, FIXW), f32)
        for d in range(1, DEGREE + 1):
            j = lob[:, None] + np.arange(W)[None, :]
            den2 = (k64[np.minimum(j + d + 1, NKNOT + 3)]
                    - k64[np.minimum(j + 1, NKNOT + 3)])
            frozen = j >= NKNOT - d
            with np.errstate(divide="ignore"):
                val = -g[d, j] / (np.where(den2 == 0, 1.0, den2) * g[d - 1, j + 1])
            a2[d - 1] = np.where((den2 == 0) | frozen, 0.0, val).astype(f32)

            fb = np.arange(NBLK - FIXNB, NBLK)
            jf = lob[fb][:, None] + np.arange(W)[None, :]
            fz = jf >= NKNOT - d
            kff[d - 1, 1] = fz.astype(f32).reshape(FIXW)
            kff[d - 1, 0] = np.where(
                fz, k64[np.minimum(jf + d, NKNOT + 3)] - C[gb[fb]][:, None], 0.0
            ).astype(f32).reshape(FIXW)
            jall = lob[:NBLK - FIXNB][:, None] + np.arange(W)[None, :]
            if (jall >= NKNOT - d).any():
                raise ValueError("frozen cols outside fix region")
            den1 = k64[np.minimum(j + d, NKNOT + 3)] - k64[j]
            if ((den1 == 0) & (j > 2)).any():
                raise ValueError("tied interior knots unsupported")

        sp_ = spans[gb]
        Q = (sp_ - lob[:, None]).astype(np.int64)
        Q = np.where((Q < 0) | (Q >= W), -1000, Q)
        qtab = np.ascontiguousarray(Q.T.astype(f32))           # [BLK, NBLK]

        # gauged coefficients, padded to WG rows per block; rows W..WG-1 are
        # zero so garbage in the matching (unwritten) lhsT rows cancels
        j = (lob[:, None] + np.arange(W)[None, :]).reshape(-1)
        cw = (c64[:, j] / g[DEGREE, j]).T.astype(f32)          # [NBLK*W, DIM]
        ctile = np.zeros((NBLK * WG, DIM), f32)
        ctile.reshape(NBLK, WG, DIM)[:, :W, :] = cw.reshape(NBLK, W, DIM)

        widx = np.tile(np.arange(W, dtype=f32), NBLK)
        # flat per-degree tables so every DVE operand is a 2-D contiguous AP
        # (3-D/strided APs eat the walrus per-instruction sync budget).
        # Order groups tables by first use so the DMA pieces pipeline:
        # [widx, negk1 | negk2_1, a2_1 | negk2_2, a2_2 | negk2_3, a2_3 | kff]
        negk1 = nkx[:, 0:W].reshape(-1)
        negk2 = [nkx[:, d + 1:d + 1 + W].reshape(-1) for d in (1, 2, 3)]
        a2f = [a2[d - 1].reshape(-1) for d in (1, 2, 3)]
        tabs = np.concatenate(
            [widx, negk1,
             negk2[0], a2f[0], negk2[1], a2f[1], negk2[2], a2f[2],
             kff.reshape(-1)]
        ).astype(f32).reshape(1, -1)
        ucol = (np.arange(BLK, dtype=f32) - 64.0).reshape(BLK, 1)
        pcol = np.ascontiguousarray(np.concatenate([ucol, qtab], axis=1))

        maps.append({
            "ctile": ctile,
            "tabs": tabs,
            "pcol": pcol,
        })
    return maps


# ----------------------------------------------------------------------------
# device program
# ----------------------------------------------------------------------------
def _build_nc():
    from contextlib import ExitStack

    import concourse.bass as bass
    import concourse.mybir as mybir
    import concourse.tile as tile
    from concourse import bacc
    from concourse.masks import make_identity

    FP = mybir.dt.float32
    AL = mybir.AluOpType

    WF = NBLK * W
    TABW = 8 * WF + DEGREE * 2 * FIXW

    FPR = mybir.dt.float32r if USE_F32R else FP

    nc = bacc.Bacc()
    y_d = nc.dram_tensor("y", [SC, DIM], FP, kind="ExternalOutput")
    ctile_d = nc.dram_tensor("ctile", [NBLK * WG, DIM], FPR, kind="ExternalInput")
    tabs_d = nc.dram_tensor("tabs", [1, TABW], FP, kind="ExternalInput")
    pcol_d = nc.dram_tensor("pcol", [BLK, 1 + NBLK], FP, kind="ExternalInput")

    with ExitStack() as ctx:
        tc = ctx.enter_context(tile.TileContext(nc))
        const = ctx.enter_context(tc.tile_pool(name="const", bufs=1))
        bwide = ctx.enter_context(tc.tile_pool(name="bwide", bufs=1))
        b3p = ctx.enter_context(tc.tile_pool(name="b3", bufs=NCHUNK + 1))
        w2tp = ctx.enter_context(tc.tile_pool(name="w2tp", bufs=1))
        tmpp = ctx.enter_context(tc.tile_pool(name="tmpp", bufs=2))
        lhsp = ctx.enter_context(tc.tile_pool(name="lhsp", bufs=2 * NCHUNK))
        outp = ctx.enter_context(tc.tile_pool(name="outp", bufs=NBLK))
        pst = ctx.enter_context(tc.tile_pool(name="pst", bufs=2, space="PSUM"))
        psmm = ctx.enter_context(tc.tile_pool(name="psmm", bufs=5, space="PSUM"))

        ident = const.tile([BLK, BLK], FP)
        make_identity(nc, ident)

        # per-partition columns: u' and the span index for the one-hot
        pcol = const.tile([BLK, 1 + NBLK], FP, tag="pcol")
        nc.sync.dma_start(pcol, pcol_d[:, :])
        ucol = pcol[:, 0:1]
        qtab = pcol[:, 1:1 + NBLK]

        # per-column tables, broadcast-DMA'd in pieces ordered by first use;
        # later pieces are emitted inside the chunk loop so the first ones
        # get the DMA queues to themselves
        tabsb = const.tile([BLK, TABW], FP, tag="tabsb")

        def load_piece(off, ln):
            nc.sync.dma_start(
                tabsb[:, off:off + ln],
                tabs_d[:, off:off + ln].to_broadcast([BLK, ln]))

        load_piece(0, WF)                  # widx (gates the init one-hot)
        load_piece(WF, WF)                 # negk1
        load_piece(2 * WF, 2 * WF)         # d1 tables

        def tabrow(i):  # flat [BLK, WF] table row i
            return tabsb[:, i * WF:(i + 1) * WF]

        widx = tabrow(0)
        negk1 = tabrow(1)
        negk2 = [tabrow(2 * d) for d in (1, 2, 3)]      # negk2[d-1]
        a2 = [tabrow(2 * d + 1) for d in (1, 2, 3)]     # a2[d-1]
        KO = 8 * WF
        kf = [tabsb[:, KO + (d - 1) * 2 * FIXW:][:, 0:FIXW] for d in (1, 2, 3)]
        ff = [tabsb[:, KO + (d - 1) * 2 * FIXW + FIXW:][:, 0:FIXW]
              for d in (1, 2, 3)]

        # coefficient windows, per chunk: 4 blocks stacked on partitions
        # (partition 32*j + w holds block 4*g+j, col w) so the 4 matmuls of
        # a group can run row-packed at tile_position=(32j, 0).  The DMAs
        # are emitted inside the chunk loop so the startup table DMAs get
        # the DMA queues first.
        csb = [None] * NCHUNK
        ctile3 = ctile_d[:, :].rearrange("(g j w) d -> (j w) g d", j=GJ, w=WG)

        # gauged-basis ping/pong, [128, NCHUNK*(CF+1)] with a zero pad col
        # per chunk (never written after memset; serves the last shift read)
        wideA = bwide.tile([BLK, NCHUNK * CFP], FP, tag="wideA")
        wideB = bwide.tile([BLK, NCHUNK * CFP], FP, tag="wideB")
        # only the per-chunk pad cols need zeroing; the rest is written
        for wt in (wideA, wideB):
            nc.gpsimd.memset(
                wt[:, :].rearrange("p (c f) -> p c f", f=CFP)[:, :, CF:CFP],
                0.0)

        # PE-side absorber: soak up the gpsimd(identity) wait once
        psd = ctx.enter_context(tc.tile_pool(name="psd", bufs=1, space="PSUM"))
        scrp = psd.tile([BLK, BLK], FP, tag="scrp")
        nc.tensor.transpose(scrp[:, :], ident[:, :], ident[:, :])

        def wslice(tile_, c, shift=0):   # flat [BLK, CF] chunk view
            base = c * CFP + shift
            return tile_[:, base:base + CF]

        def pslice(tile_, pr, shift=0):  # paired [BLK, 2, CF] chunk-pair view
            base = pr * 2 * CFP
            return tile_[:, base:base + 2 * CFP].rearrange(
                "p (c f) -> p c f", c=2)[:, :, shift:shift + CF]

        def ptab(tab, pr):               # paired table view [BLK, 2, CF]
            return tab[:, pr * 2 * CF:(pr + 1) * 2 * CF].rearrange(
                "p (c f) -> p c f", c=2)

        # ---- pair-major: recursion for a chunk pair, then its GEMM ----
        for pr in range(2):
            for ci in range(2):
                cidx = 2 * pr + ci
                csl = slice(cidx * CF, (cidx + 1) * CF)
                bsl = slice(cidx * CB, (cidx + 1) * CB)
                nc.vector.tensor_tensor(
                    wslice(wideA, cidx).rearrange("p (b w) -> p b w", w=W),
                    widx[:, csl].rearrange("p (b w) -> p b w", w=W),
                    qtab[:, bsl, None].to_broadcast([BLK, CB, W]),
                    AL.is_equal)
            src_, dst = wideA, wideB
            for d in (1, 2):
                out = pslice(dst, pr)
                nc.vector.scalar_tensor_tensor(
                    out, ptab(negk1, pr), ucol, pslice(src_, pr),
                    AL.add, AL.mult)
                w2t = w2tp.tile([BLK, 2, CF], FP, tag="w2t")
                nc.vector.scalar_tensor_tensor(
                    w2t[:, :, :], ptab(negk2[d - 1], pr), ucol,
                    ptab(a2[d - 1], pr), AL.add, AL.mult)
                nc.vector.tensor_tensor(
                    w2t[:, :, :], w2t[:, :, :], pslice(src_, pr, shift=1),
                    AL.mult)
                nc.vector.tensor_tensor(out, out, w2t[:, :, :], AL.add)
                if pr == 0:
                    if d == 1:
                        load_piece(4 * WF, 2 * WF)   # d2 tables
                    else:
                        load_piece(6 * WF, 2 * WF)   # d3 tables
                        load_piece(8 * WF, DEGREE * 2 * FIXW)  # kff
                # stale-column fix for degree d (last chunk's tail only)
                if pr == 1:
                    ntmp = tmpp.tile([BLK, FIXW], FP, tag="ntmp")
                    nc.vector.scalar_tensor_tensor(
                        ntmp[:, :], kf[d - 1], ucol,
                        wslice(src_, 3)[:, CF - FIXW:CF],
                        AL.subtract, AL.mult)
                    nc.vector.tensor_tensor(
                        ntmp[:, :], ntmp[:, :], ff[d - 1], AL.mult)
                    fout = out[:, 1, CF - FIXW:CF]
                    nc.vector.tensor_tensor(fout, fout, ntmp[:, :], AL.add)
                src_, dst = dst, src_

            for ci in range(2):
                cidx = 2 * pr + ci
                ct = const.tile([BLK, NGRP, DIM], FPR, tag=f"csb{cidx}")
                nc.sync.dma_start(
                    ct, ctile3[:, cidx * NGRP:(cidx + 1) * NGRP, :])
                csb[cidx] = ct
                nc.tensor.matmul(
                    scrp[:, 0:1], ct[:, 0, 0:BLK].bitcast(FP),
                    ct[:, 0, 0:1].bitcast(FP), start=True, stop=True)

            # d3 (into the padded GEMM layout) + GEMM for this pair's chunks
            for ci in range(2):
                cidx = 2 * pr + ci
                csl = slice(cidx * CF, (cidx + 1) * CF)
                b3t = b3p.tile([BLK, CG], FP, tag="b3")
                b3v = b3t[:, :].rearrange("p (b wg) -> p b wg", wg=WG)
                nc.gpsimd.memset(b3v[:, :, W:WG], 0.0)
                out = b3v[:, :, 0:W]

                def f3(ap):
                    return ap.rearrange("p (b w) -> p b w", w=W)

                nc.vector.scalar_tensor_tensor(
                    out, f3(negk1[:, csl]), ucol, f3(wslice(wideA, cidx)),
                    AL.add, AL.mult)
                w2t = w2tp.tile([BLK, CF], FP, tag="w2t3")
                nc.vector.scalar_tensor_tensor(
                    w2t[:, :], negk2[2][:, csl], ucol, a2[2][:, csl],
                    AL.add, AL.mult)
                nc.vector.tensor_tensor(
                    w2t[:, :], w2t[:, :], wslice(wideA, cidx, shift=1),
                    AL.mult)
                nc.vector.tensor_tensor(out, out, f3(w2t[:, :]), AL.add)
                if cidx == NCHUNK - 1:
                    ntmp = tmpp.tile([BLK, FIXW], FP, tag="ntmp")
                    nc.vector.scalar_tensor_tensor(
                        ntmp[:, :], kf[2], ucol,
                        wslice(wideA, cidx)[:, CF - FIXW:CF],
                        AL.subtract, AL.mult)
                    nc.vector.tensor_tensor(
                        ntmp[:, :], ntmp[:, :], ff[2], AL.mult)
                    fout = out[:, CB - FIXNB:CB, :]
                    nc.vector.tensor_tensor(
                        fout, fout, f3(ntmp[:, :]), AL.add)

                for gg in range(NGRP):
                    tps = pst.tile([BLK, BLK], FP, tag="tps")
                    nc.tensor.transpose(
                        tps[:, :], b3t[:, gg * GJ * WG:(gg + 1) * GJ * WG],
                        ident[:, :])
                    lh4 = lhsp.tile([BLK, BLK], FPR, tag="lh4")
                    nc.scalar.copy(lh4[:, :], tps[:, :])
                    for j in range(GJ):
                        gb = cidx * CB + gg * GJ + j
                        ps = psmm.tile([BLK, DIM], FP, tag="ps")
                        nc.tensor.matmul(
                            ps[:, :], lh4[32 * j:32 * (j + 1), :],
                            csb[cidx][32 * j:32 * (j + 1), gg, :],
                            start=True, stop=True, tile_position=(32 * j, 0))
                        ob = outp.tile([BLK, DIM], FP, tag="ob")
                        if pr == 1 and j % 2 == 0:
                            # DVE is past the recursion by now; share load
                            nc.vector.tensor_copy(ob[:, :], ps[:, :])
                        else:
                            nc.scalar.copy(ob[:, :], ps[:, :])
                        nc.sync.dma_start(y_d[gb * BLK:(gb + 1) * BLK, :],
                                          ob[:, :])

    nc.compile()
    return nc


_NC_CACHE = {}


def _get_nc():
    if "nc" not in _NC_CACHE:
        _NC_CACHE["nc"] = _build_nc()
    return _NC_CACHE["nc"]


# ----------------------------------------------------------------------------
# entry point
# ----------------------------------------------------------------------------
def kernel(t, c, delta):
    t = np.asarray(t, np.float32)
    c = np.asarray(c, np.float32)
    d = np.asarray(delta).reshape(()).item()

    t_max = float(t[-1])
    sample_points = np.arange(0.0, t_max, float(d), dtype=np.float32)
    S = sample_points.shape[0]

    if (S != S_TOTAL or t.shape != (NKNOT,) or c.shape != (DIM, NKNOT)):
        return _numpy_reference(t, c, d)

    try:
        maps = _host_prep(t, c)
    except ValueError:
        # pathological knot layout for the windowed scheme
        return _numpy_reference(t, c, d)

    from concourse.bass_utils import run_bass_kernel_spmd

    nc = _get_nc()
    res = run_bass_kernel_spmd(nc, maps, core_ids=list(range(NCORE)))
    out = np.concatenate([res.results[m]["y"] for m in range(NCORE)], axis=0)
    return sample_points, out


if __name__ == "__main__":
    import jax
    jax.config.update("jax_platforms", "cpu")
    rng = np.random.default_rng(0)
    # smoke-test the numpy fallback against itself via host tables + model
    print("kernel.py loaded OK")
